# revision 1
# baseline (speedup 1.0000x reference)
"""Trainium2 Bass kernel: per-element golden-section search maximizing the
lognormal-CDF surplus  s(d) = bid*(1-d)*Phi((ln(d*bid)-mu)/sigma).

Mathematical restructuring (exact in real arithmetic, validated in fp32):
  - After k iterations the golden-section interval is [bmin, bmin+c_k] with
    c_k = phi^-k data-independent, so only bmin is tracked per element.
    Probes are d1 = bmin+c_{k+2}, d2 = bmin+c_{k+1}; the +c folds into the
    Ln activation's free input bias.
  - Comparison s1 > s2 is computed as s_i = (erf(z_i)+1) * M_i (one fused
    scalar_tensor_tensor per probe), M_i = 1-d_i, dropping the common
    positive factor bid*0.5.  The erf-saturation tie (both CDFs underflow
    to 0) then gives s1 = s2 = 0 exactly, matching the reference branch
    (cond False -> step right) deterministically via  step = (delta<=0)*c_{k+2}.
  - z_i = (ln(d_i) + A2) * B with A2 = ln(bid)-mu, B = 1/(sigma*sqrt(2))
    computed once per element (B via exp(-ln(sigma*sqrt2)) on ACT).

Engine mapping per iteration (per [128, FD] chunk):
  ACT: 2x Ln (bias=c), 2x Erf, 2x Copy (M_i = -bmin + (1-c))   [2 table swaps]
  VE : 4x tensor_tensor (z affine), 2x scalar_tensor_tensor (s_i),
       1x sub (delta), 1x tensor_scalar dual-op (step, 2x mode), 1x add (bmin)
Chunks are processed in interleaved pairs so ACT work on one chunk hides
under VE work on the other.
"""
import sys

sys.path.insert(0, "/opt/trn_rl_repo")

import numpy as np

N_TOTAL = 16777216
N_CORES = 8
N_PER_CORE = N_TOTAL // N_CORES  # 2097152
P = 128
FD = 2048
N_ITER = 20
GR = (np.sqrt(5.0) + 1.0) / 2.0


def _build_nc(n_per_core, fd, group_size=2):
    import concourse.bass as bass
    import concourse.bacc as bacc
    import concourse.mybir as mybir
    import concourse.tile as tile

    AF = mybir.ActivationFunctionType
    ALU = mybir.AluOpType
    dt = mybir.dt.float32

    n_chunks = n_per_core // (P * fd)
    assert n_chunks * P * fd == n_per_core

    # fp64-computed interval constants, used as fp32 immediates
    c = [GR ** (-k) for k in range(N_ITER + 4)]

    nc = bacc.Bacc(None, target_bir_lowering=False)

    # Ln-activation biases (c_k interval constants) must exist as const APs.
    def register_const(value: float):
        if (dt, value) in nc.const_aps.aps:
            return
        t = nc.alloc_sbuf_tensor(f"const-f32-c{len(nc.const_aps.aps)}", [128, 1], dt)
        nc.gpsimd.memset(t.ap(), value)
        nc.const_aps.aps[(dt, value)] = t.ap()

    for k in range(1, N_ITER + 2):
        register_const(float(c[k]))
    nc.all_engine_barrier()

    params = nc.declare_dram_parameter("params", [n_per_core, 2], dt, isOutput=False)
    bids = nc.declare_dram_parameter("bids", [n_per_core], dt, isOutput=False)
    out = nc.declare_dram_parameter("out", [n_per_core], dt, isOutput=True)

    # contiguous [G, 128, 2*fd] view of interleaved (mu, sigma) pairs
    params_v = params.rearrange("(g p f) c -> g p (f c)", p=P, f=fd)
    bids_v = bids.rearrange("(g p f) -> g p f", p=P, f=fd)
    out_v = out.rearrange("(g p f) -> g p f", p=P, f=fd)

    sqrt2 = float(np.sqrt(2.0))

    with tile.TileContext(nc) as tc:
        with (
            tc.tile_pool(name="st_bmin", bufs=2 * group_size) as p_bmin,
            tc.tile_pool(name="st_a2", bufs=2 * group_size) as p_a2,
            tc.tile_pool(name="st_b", bufs=2 * group_size) as p_b,
            tc.tile_pool(name="t1", bufs=group_size + 1) as p_t1,
            tc.tile_pool(name="t2", bufs=group_size + 1) as p_t2,
            tc.tile_pool(name="t3", bufs=group_size) as p_t3,
            tc.tile_pool(name="t4", bufs=group_size) as p_t4,
            tc.tile_pool(name="pload", bufs=group_size) as p_pl,
        ):
            for g0 in range(0, n_chunks, group_size):
                members = []
                # ---- setup each chunk of the group ----
                for gi in range(g0, min(g0 + group_size, n_chunks)):
                    bmin = p_bmin.tile([P, fd], dt, tag="bmin")
                    a2 = p_a2.tile([P, fd], dt, tag="a2")
                    bt = p_b.tile([P, fd], dt, tag="b")
                    # load bid -> a2 (scratch), then logbid in place
                    nc.sync.dma_start(a2[:], bids_v[gi])
                    nc.scalar.activation(a2[:], a2[:], AF.Ln)
                    # interleaved params arrive in two [P, fd] staging halves
                    for h in range(2):
                        pl = p_pl.tile([P, fd], dt, tag="pl")
                        nc.sync.dma_start(pl[:], params_v[gi, :, h * fd:(h + 1) * fd])
                        plv = pl.rearrange("p (f c) -> p f c", c=2)
                        half = slice(h * (fd // 2), (h + 1) * (fd // 2))
                        # B = ln(sigma*sqrt2); sigma read strided from pl
                        nc.scalar.activation(bt[:, half], plv[:, :, 1], AF.Ln, scale=sqrt2)
                        # A2 = logbid - mu; mu read strided from pl
                        nc.vector.tensor_sub(a2[:, half], a2[:, half], plv[:, :, 0])
                    # B = exp(-B)
                    nc.scalar.activation(bt[:], bt[:], AF.Exp, scale=-1.0)
                    # bmin <- 0
                    nc.gpsimd.memset(bmin[:], 0.0)
                    members.append((gi, bmin, a2, bt))

                scratch = {}
                for k in range(N_ITER):
                    c1, c2 = float(c[k + 1]), float(c[k + 2])
                    # stage Ln  (one table residency)
                    for gi, bmin, a2, bt in members:
                        t1 = p_t1.tile([P, fd], dt, tag="t1")
                        t2 = p_t2.tile([P, fd], dt, tag="t2")
                        scratch[gi] = (t1, t2)
                        nc.scalar.activation(t1[:], bmin[:], AF.Ln, bias=c2)
                        nc.scalar.activation(t2[:], bmin[:], AF.Ln, bias=c1)
                    # stage z = (L + A2) * B
                    for gi, bmin, a2, bt in members:
                        t1, t2 = scratch[gi]
                        nc.vector.tensor_add(t1[:], t1[:], a2[:])
                        nc.vector.tensor_mul(t1[:], t1[:], bt[:])
                        nc.vector.tensor_add(t2[:], t2[:], a2[:])
                        nc.vector.tensor_mul(t2[:], t2[:], bt[:])
                    # stage Erf (one table residency)
                    for gi, bmin, a2, bt in members:
                        t1, t2 = scratch[gi]
                        nc.scalar.activation(t1[:], t1[:], AF.Erf)
                        nc.scalar.activation(t2[:], t2[:], AF.Erf)
                    # stage M (Copy: in every table set)
                    for gi, bmin, a2, bt in members:
                        t1, t2 = scratch[gi]
                        t3 = p_t3.tile([P, fd], dt, tag="t3")
                        t4 = p_t4.tile([P, fd], dt, tag="t4")
                        scratch[gi] = (t1, t2, t3, t4)
                        nc.scalar.activation(
                            t3[:], bmin[:], AF.Copy, scale=-1.0, bias=1.0 - c2
                        )
                        nc.scalar.activation(
                            t4[:], bmin[:], AF.Copy, scale=-1.0, bias=1.0 - c1
                        )
                    # stage s / delta / step / update
                    for gi, bmin, a2, bt in members:
                        t1, t2, t3, t4 = scratch[gi]
                        nc.vector.scalar_tensor_tensor(
                            t1[:], t1[:], 1.0, t3[:], op0=ALU.add, op1=ALU.mult
                        )
                        nc.vector.scalar_tensor_tensor(
                            t2[:], t2[:], 1.0, t4[:], op0=ALU.add, op1=ALU.mult
                        )
                        nc.vector.tensor_sub(t1[:], t1[:], t2[:])
                        nc.vector.tensor_scalar(
                            t1[:], t1[:], 0.0, c2, op0=ALU.is_le, op1=ALU.mult
                        )
                        nc.vector.tensor_add(bmin[:], bmin[:], t1[:])

                # ---- finish: midpoint, store ----
                half_w = float(c[N_ITER] / 2.0)
                for gi, bmin, a2, bt in members:
                    t1 = p_t1.tile([P, fd], dt, tag="t1")
                    nc.scalar.activation(t1[:], bmin[:], AF.Copy, scale=1.0, bias=half_w)
                    nc.sync.dma_start(out_v[gi], t1[:])

    nc.finalize()
    return nc


_CACHED = {}


def _get_nc(n_per_core, fd, group_size=2):
    key = (n_per_core, fd, group_size)
    if key not in _CACHED:
        _CACHED[key] = _build_nc(n_per_core, fd, group_size)
    return _CACHED[key]


def kernel(params: np.ndarray, bid_prices: np.ndarray) -> np.ndarray:
    from concourse.bass_utils import run_bass_kernel_spmd

    params = np.ascontiguousarray(params, dtype=np.float32)
    bid_prices = np.ascontiguousarray(bid_prices, dtype=np.float32)
    n = bid_prices.shape[0]
    n_per_core = n // N_CORES

    nc = _get_nc(n_per_core, FD)

    in_maps = []
    for i in range(N_CORES):
        sl = slice(i * n_per_core, (i + 1) * n_per_core)
        in_maps.append({"params": params[sl], "bids": bid_prices[sl]})

    res = run_bass_kernel_spmd(nc, in_maps, core_ids=list(range(N_CORES)))
    return np.concatenate([r["out"] for r in res.results], axis=0)


if __name__ == "__main__":
    # smoke test with random data
    rng = np.random.RandomState(0)
    n = N_TOTAL
    params = np.stack(
        [rng.randn(n).astype(np.float32),
         rng.uniform(0.2, 1.5, n).astype(np.float32)], axis=-1
    )
    bids = rng.uniform(0.1, 10.0, n).astype(np.float32)
    out = kernel(params=params, bid_prices=bids)
    print("out", out.shape, out.dtype, out[:8])



# revision 4
# speedup vs baseline: 1.6369x; 1.6369x over previous
"""Trainium2 Bass kernel: per-element maximization of the lognormal-CDF
surplus  s(d) = bid*(1-d)*Phi((ln(d*bid)-mu)/sigma),  d in [0,1].

Algorithm: the reference runs 20 golden-section iterations on s(d) (two
surplus evaluations per iteration).  s is log-concave in d (product of
log-concave factors composed with concave increasing maps), so s' crosses
zero exactly once and the argmax can instead be found by BISECTION ON THE
SIGN OF s'(d) - one evaluation per iteration, 0.5x interval shrink per
iteration (vs 0.618x for GSS):

  s'(d) >= 0  <=>  (1-d) * B * phi0 * e^{-z^2} >= d * (1 + erf z)
     z = (ln d - m) * B,  m = mu - ln bid,  B = 1/(sigma*sqrt2),
     phi0 = 2/sqrt(pi)

Validated against the reference output: rel-L2 plateaus at ~5.2e-3 for
K >= 11 (the residual is reference fp32 (1+erf) quantization noise on
deep-tail elements, not bisection resolution) - comfortably under the
2e-2 gate, and robust to 1e-3 activation-table error.

Implementation notes (per [128, FD] chunk):
  - Only the interval MIDPOINT D is tracked:  D' = D + (delta>=0)*w - w/2,
    one fused custom-DVE op (GSS_STEPD); the final STEPD emits the answer.
  - z' = (ln D - m)/sigma; erf(z) via ACT Erf with input scale 1/sqrt2;
    the gaussian side folds B*phi0 into the ACT Exp bias:
      P' = Exp(-(0.5 z'^2 + ln sigma) + ln(phi0/sqrt2)) = B*phi0*e^{-z^2}
    so the per-iter ACT tables are {Ln, Exp} (one set) + {Erf}: 2 swaps.
  - Custom fused DVE ops (registered at import): sq-scale-add for the Exp
    argument, the step update, and iteration-0 specializations (D_0 = 0.5
    is a compile-time constant, so iter 0 needs no Ln and folds M=0.5
    into the Exp bias).
  - Work is split DVE / Pool (gpsimd) / ACT; the delta & p_r ops alternate
    between DVE and Pool by chunk parity to balance the two queues.
"""
import sys

sys.path.insert(0, "/opt/trn_rl_repo")

import numpy as np

N_TOTAL = 16777216
N_CORES = 8
N_PER_CORE = N_TOTAL // N_CORES  # 2097152
P = 128
FD = 1024
GROUP = 3
K_ITERS = 11

LN_HALF = float(np.log(0.5))
INV_SQRT2 = float(1.0 / np.sqrt(2.0))
# ln(phi0/sqrt2), phi0 = 2/sqrt(pi)
LNPHI = float(np.log(2.0 / np.sqrt(np.pi)) - 0.5 * np.log(2.0))
LNPHI_HALF = float(LNPHI + np.log(0.5))  # iter-0: M = 0.5 folded in

_ops_registered = {}


def _register_ops():
    """Register the fused custom-DVE ops (documented extension point:
    dve_ops.OPS + _SUB_OPCODE_FOR_NAME + CUSTOM_DVE_SPECS). uops_sha is
    computed here the same way DveOp.compile derives it."""
    if _ops_registered:
        return _ops_registered
    import concourse.dve_ops as dve_ops
    from concourse.dve_ops import DveOp, OPS
    from concourse.dve_spec import Spec, Src0, Src1, C0, C2, Zero, sq, lower
    from concourse.dve_spec import _has_src1 as has_src1
    from concourse.dve_uop import DveOpSpec

    def ref_sqsa(in0, in1, s0, s1, imm2):
        return (np.float32(s0) * in0.astype(np.float32) ** 2 + in1).astype(np.float32)

    def ref_stepd(in0, in1, s0, s1, imm2):
        return (in1 + (in0 >= 0).astype(np.float32) * np.float32(imm2)
                + np.float32(s0)).astype(np.float32)

    def ref_z0(in0, in1, s0, s1, imm2):
        return ((np.float32(s0) - in0.astype(np.float32)) * in1).astype(np.float32)

    def ref_step0(in0, in1, s0, s1, imm2):
        return ((in0 >= 0).astype(np.float32) * np.float32(imm2)
                + np.float32(s0)).astype(np.float32)

    defs = [
        # v2 = 0.5*z'^2 + ln(sigma)
        ("GSS_SQSA", sq(Src0) * C0 + Src1, ref_sqsa),
        # D' = D + (delta >= 0)*w - w/2
        ("GSS_STEPD", Src1 + (Src0 >= Zero) * C2 + C0, ref_stepd),
        # z0 = (ln(0.5) - m) * (1/sigma)
        ("GSS_Z0", (C0 - Src0) * Src1, ref_z0),
        # D1 = (delta >= 0)*0.5 + 0.25
        ("GSS_STEP0", (Src0 >= Zero) * C2 + C0, ref_step0),
    ]
    for name, body, ref in defs:
        if name in dve_ops._SUB_OPCODE_FOR_NAME:
            _ops_registered[name] = next(o for o in OPS if o.name == name)
            continue
        row = dve_ops._CUSTOM_DVE_ROW_BASE + len(OPS)
        assert row < 0x20
        spec = Spec(body=body, reference=ref)
        shas = {}
        for ver in ("v3", "v4"):
            uops = lower(spec, ver=ver)
            shas[ver] = DveOpSpec(
                name=name, opcode=row, uops=uops, rd1_en=has_src1(spec)
            ).sha(ver)
        op = DveOp(name, spec, subdim=False, uops_sha=shas)
        OPS.append(op)
        dve_ops._SUB_OPCODE_FOR_NAME[name] = row
        dve_ops.CUSTOM_DVE_SPECS[name] = spec
        _ops_registered[name] = op
    return _ops_registered


def _build_nc(n_per_core, fd, group_size):
    import concourse.bass as bass  # noqa: F401
    import concourse.bacc as bacc
    import concourse.mybir as mybir
    import concourse.tile as tile

    ops = _register_ops()
    SQSA, STEPD, Z0, STEP0 = (
        ops["GSS_SQSA"], ops["GSS_STEPD"], ops["GSS_Z0"], ops["GSS_STEP0"]
    )

    AF = mybir.ActivationFunctionType
    ALU = mybir.AluOpType
    dt = mybir.dt.float32

    n_chunks = n_per_core // (P * fd)
    assert n_chunks * P * fd == n_per_core

    nc = bacc.Bacc(None, target_bir_lowering=False)

    def register_const(value: float):
        if (dt, value) in nc.const_aps.aps:
            return
        t = nc.alloc_sbuf_tensor(f"const-f32-c{len(nc.const_aps.aps)}", [128, 1], dt)
        nc.gpsimd.memset(t.ap(), value)
        nc.const_aps.aps[(dt, value)] = t.ap()

    for v in (0.0, LNPHI, LNPHI_HALF):
        register_const(float(v))
    nc.all_engine_barrier()

    params = nc.declare_dram_parameter("params", [n_per_core, 2], dt, isOutput=False)
    bids = nc.declare_dram_parameter("bids", [n_per_core], dt, isOutput=False)
    out = nc.declare_dram_parameter("out", [n_per_core], dt, isOutput=True)

    params_v = params.rearrange("(g p f) c -> g p (f c)", p=P, f=fd)
    bids_v = bids.rearrange("(g p f) -> g p f", p=P, f=fd)
    out_v = out.rearrange("(g p f) -> g p f", p=P, f=fd)

    with tile.TileContext(nc) as tc:
        with (
            tc.tile_pool(name="st_d", bufs=2 * group_size) as p_d,
            tc.tile_pool(name="st_m", bufs=2 * group_size) as p_m,
            tc.tile_pool(name="st_rs", bufs=2 * group_size) as p_rs,
            tc.tile_pool(name="st_ls", bufs=2 * group_size) as p_ls,
            tc.tile_pool(name="s1", bufs=group_size + 1) as p_s1,
            tc.tile_pool(name="s2", bufs=group_size + 1) as p_s2,
            tc.tile_pool(name="s3", bufs=group_size + 1) as p_s3,
            tc.tile_pool(name="s4", bufs=group_size + 1) as p_s4,
            tc.tile_pool(name="pload", bufs=2) as p_pl,
        ):
            for g0 in range(0, n_chunks, group_size):
                members = []
                # ---- per-chunk setup ----
                for gi in range(g0, min(g0 + group_size, n_chunks)):
                    D = p_d.tile([P, fd], dt, tag="D")
                    m = p_m.tile([P, fd], dt, tag="m")
                    rs = p_rs.tile([P, fd], dt, tag="rs")
                    ls = p_ls.tile([P, fd], dt, tag="ls")
                    # bid -> m (staging), then ln in place
                    nc.sync.dma_start(m[:], bids_v[gi])
                    nc.scalar.activation(m[:], m[:], AF.Ln)
                    for h in range(2):
                        pl = p_pl.tile([P, fd], dt, tag="pl")
                        nc.sync.dma_start(pl[:], params_v[gi, :, h * fd:(h + 1) * fd])
                        plv = pl.rearrange("p (f c) -> p f c", c=2)
                        half = slice(h * (fd // 2), (h + 1) * (fd // 2))
                        # ls = ln(sigma); rs = 1/sigma; m = mu - ln(bid)
                        nc.scalar.activation(ls[:, half], plv[:, :, 1], AF.Ln)
                        nc.vector.reciprocal_approx_fast(
                            out=rs[:, half], in_=plv[:, :, 1]
                        )
                        nc.vector.tensor_sub(m[:, half], plv[:, :, 0], m[:, half])
                    members.append((gi, D, m, rs, ls))

                # ---- iteration 0: D_0 = 0.5 (compile-time constant) ----
                scratch = {}
                for gi, D, m, rs, ls in members:
                    s1 = p_s1.tile([P, fd], dt, tag="s1")
                    s2 = p_s2.tile([P, fd], dt, tag="s2")
                    s3 = p_s3.tile([P, fd], dt, tag="s3")
                    s4 = p_s4.tile([P, fd], dt, tag="s4")
                    scratch[gi] = (s1, s2, s3, s4)
                    nc.vector._custom_dve(Z0, out=s1[:], in0=m[:], in1=rs[:],
                                          s0=LN_HALF)
                for gi, D, m, rs, ls in members:
                    s1, s2, s3, s4 = scratch[gi]
                    nc.scalar.activation(s4[:], s1[:], AF.Erf, scale=INV_SQRT2)
                for gi, D, m, rs, ls in members:
                    s1, s2, s3, s4 = scratch[gi]
                    nc.vector._custom_dve(SQSA, out=s2[:], in0=s1[:], in1=ls[:],
                                          s0=0.5)
                for gi, D, m, rs, ls in members:
                    s1, s2, s3, s4 = scratch[gi]
                    # p_l = P' * 0.5 (M folded into bias)
                    nc.scalar.activation(s3[:], s2[:], AF.Exp, scale=-1.0,
                                         bias=LNPHI_HALF)
                for gi, D, m, rs, ls in members:
                    s1, s2, s3, s4 = scratch[gi]
                    # p_r = (E+1)*0.5
                    nc.vector.tensor_scalar(s4[:], s4[:], 1.0, 0.5,
                                            op0=ALU.add, op1=ALU.mult)
                for gi, D, m, rs, ls in members:
                    s1, s2, s3, s4 = scratch[gi]
                    nc.gpsimd.tensor_sub(s4[:], s3[:], s4[:])
                for gi, D, m, rs, ls in members:
                    s1, s2, s3, s4 = scratch[gi]
                    nc.vector._custom_dve(STEP0, out=D[:], in0=s4[:],
                                          s0=0.25, imm2=0.5)

                # ---- iterations 1..K-1 ----
                for k in range(1, K_ITERS):
                    w = float(2.0 ** -(k + 1))
                    s0 = float(-(2.0 ** -(k + 2)))
                    for gi, D, m, rs, ls in members:
                        s1 = p_s1.tile([P, fd], dt, tag="s1")
                        s2 = p_s2.tile([P, fd], dt, tag="s2")
                        s3 = p_s3.tile([P, fd], dt, tag="s3")
                        s4 = p_s4.tile([P, fd], dt, tag="s4")
                        scratch[gi] = (s1, s2, s3, s4)
                        nc.scalar.activation(s1[:], D[:], AF.Ln)
                    for gi, D, m, rs, ls in members:
                        s1, s2, s3, s4 = scratch[gi]
                        # M = 1 - D  (Copy is in every ACT table set)
                        nc.scalar.activation(s3[:], D[:], AF.Copy, scale=-1.0,
                                             bias=1.0)
                    for gi, D, m, rs, ls in members:
                        s1, s2, s3, s4 = scratch[gi]
                        nc.gpsimd.tensor_sub(s1[:], s1[:], m[:])
                    for gi, D, m, rs, ls in members:
                        s1, s2, s3, s4 = scratch[gi]
                        nc.vector.tensor_mul(s1[:], s1[:], rs[:])
                    for gi, D, m, rs, ls in members:
                        s1, s2, s3, s4 = scratch[gi]
                        nc.scalar.activation(s4[:], s1[:], AF.Erf, scale=INV_SQRT2)
                    for gi, D, m, rs, ls in members:
                        s1, s2, s3, s4 = scratch[gi]
                        nc.vector._custom_dve(SQSA, out=s2[:], in0=s1[:],
                                              in1=ls[:], s0=0.5)
                    for gi, D, m, rs, ls in members:
                        s1, s2, s3, s4 = scratch[gi]
                        nc.scalar.activation(s2[:], s2[:], AF.Exp, scale=-1.0,
                                             bias=LNPHI)
                    for i, (gi, D, m, rs, ls) in enumerate(members):
                        s1, s2, s3, s4 = scratch[gi]
                        # p_l = P' * M  (alternates DVE/Pool to balance queues)
                        eng = nc.gpsimd if (i + k) % 2 == 0 else nc.vector
                        eng.tensor_mul(s3[:], s2[:], s3[:])
                    for gi, D, m, rs, ls in members:
                        s1, s2, s3, s4 = scratch[gi]
                        # p_r = (E+1)*D
                        nc.vector.scalar_tensor_tensor(s4[:], s4[:], 1.0, D[:],
                                                       op0=ALU.add, op1=ALU.mult)
                    for gi, D, m, rs, ls in members:
                        s1, s2, s3, s4 = scratch[gi]
                        nc.gpsimd.tensor_sub(s4[:], s3[:], s4[:])
                    for gi, D, m, rs, ls in members:
                        s1, s2, s3, s4 = scratch[gi]
                        nc.vector._custom_dve(STEPD, out=D[:], in0=s4[:],
                                              in1=D[:], s0=s0, imm2=w)

                # ---- store (the last STEPD already wrote the midpoint) ----
                for gi, D, m, rs, ls in members:
                    nc.sync.dma_start(out_v[gi], D[:])

    nc.finalize()
    return nc


_CACHED = {}


def _get_nc(n_per_core, fd=FD, group_size=GROUP):
    key = (n_per_core, fd, group_size)
    if key not in _CACHED:
        _CACHED[key] = _build_nc(n_per_core, fd, group_size)
    return _CACHED[key]


def kernel(params: np.ndarray, bid_prices: np.ndarray) -> np.ndarray:
    from concourse.bass_utils import run_bass_kernel_spmd

    params = np.ascontiguousarray(params, dtype=np.float32)
    bid_prices = np.ascontiguousarray(bid_prices, dtype=np.float32)
    n = bid_prices.shape[0]
    n_per_core = n // N_CORES

    nc = _get_nc(n_per_core)

    in_maps = []
    for i in range(N_CORES):
        sl = slice(i * n_per_core, (i + 1) * n_per_core)
        in_maps.append({"params": params[sl], "bids": bid_prices[sl]})

    res = run_bass_kernel_spmd(nc, in_maps, core_ids=list(range(N_CORES)))
    return np.concatenate([r["out"] for r in res.results], axis=0)


if __name__ == "__main__":
    rng = np.random.RandomState(0)
    n = N_TOTAL
    params = np.stack(
        [rng.randn(n).astype(np.float32),
         rng.uniform(0.2, 1.5, n).astype(np.float32)], axis=-1
    )
    bids = rng.uniform(0.1, 10.0, n).astype(np.float32)
    out = kernel(params=params, bid_prices=bids)
    print("out", out.shape, out.dtype, out[:8])
